# revision 16
# baseline (speedup 1.0000x reference)
import sys

sys.path.insert(0, "/opt/trn_rl_repo")

import numpy as np

from concourse import bacc, mybir
from concourse.tile import TileContext
from concourse.bass_utils import run_bass_kernel_spmd

P = 128          # SBUF partitions
D = 128          # embedding dim
S = 6            # 1 positive + 5 negative slots per sample
N = 65536        # total samples
V = 1_000_000    # weights table rows
N_CORES = 8
N_SH = N // N_CORES            # 8192 samples per core
NP_ = N_SH * S                 # 49152 (sample, slot) pairs per core
BK = 32768                     # weight-row bucket size (int16 offset range)
NB = (V + BK - 1) // BK        # 31 buckets

TRACE = False
LAST_EXEC_NS = None
LAST_NC = None
LAST_CAP = None


def build_nc(cap: int):
    """cap: padded pairs per bucket (multiple of 128), same for all buckets."""
    nslot = cap // 128
    ncol = cap // 16
    nc = bacc.Bacc("TRN2", num_swdge_queues=4)
    w_dram = nc.declare_dram_parameter("weights", [V, D], mybir.dt.float32, isOutput=False)
    e_dram = nc.declare_dram_parameter("e_t", [NB, P, cap], mybir.dt.float32, isOutput=False)
    i_dram = nc.declare_dram_parameter("idx_t", [NB, P, ncol], mybir.dt.int16, isOutput=False)
    o_dram = nc.declare_dram_parameter("out", [P, NB], mybir.dt.float32, isOutput=True)

    dma_engines = None  # filled inside context

    with TileContext(nc) as tc:
        with (
            tc.tile_pool(name="ip", bufs=2) as ip,
            tc.tile_pool(name="wp", bufs=3) as wp,
            tc.tile_pool(name="ep", bufs=3) as ep,
            tc.tile_pool(name="sp", bufs=3) as sp,
            tc.tile_pool(name="accp", bufs=1) as accp,
        ):
            dma_engines = [nc.sync, nc.scalar]
            acc = accp.tile([P, NB], mybir.dt.float32)
            for b in range(NB):
                rows = min(BK, V - b * BK)
                it = ip.tile([P, ncol], mybir.dt.int16)
                nc.sync.dma_start(out=it[:], in_=i_dram[b, :, :])
                et = ep.tile([P, cap], mybir.dt.float32)
                dma_engines[b % 2].dma_start(out=et[:], in_=e_dram[b, :, :])
                wt = wp.tile([P, cap], mybir.dt.float32)
                nc.gpsimd.dma_gather(
                    out_ap=wt[:].rearrange("p (j d) -> p j d", d=D),
                    in_ap=w_dram[b * BK: b * BK + rows, :],
                    idxs_ap=it[:],
                    num_idxs=cap, num_idxs_reg=cap, elem_size=D,
                    queue_num=b % 4, single_packet=False,
                )
                prod = sp.tile([P, cap], mybir.dt.float32)
                nc.vector.tensor_tensor(
                    out=prod[:], in0=wt[:], in1=et[:], op=mybir.AluOpType.mult,
                )
                logits = sp.tile([P, nslot], mybir.dt.float32)
                nc.vector.tensor_reduce(
                    out=logits[:],
                    in_=prod[:].rearrange("p (j d) -> p j d", d=D),
                    op=mybir.AluOpType.add, axis=mybir.AxisListType.X,
                )
                # softplus(x) = -ln(sigmoid(-x)); host negates the summed output.
                sg = sp.tile([P, nslot], mybir.dt.float32)
                nc.scalar.activation(
                    out=sg[:], in_=logits[:],
                    func=mybir.ActivationFunctionType.Sigmoid, scale=-1.0,
                )
                ln = sp.tile([P, nslot], mybir.dt.float32)
                nc.scalar.activation(
                    out=ln[:], in_=sg[:],
                    func=mybir.ActivationFunctionType.Ln, scale=1.0,
                    accum_out=acc[:, b:b + 1],
                )
            nc.sync.dma_start(out=o_dram[:, :], in_=acc[:])
    nc.compile()
    return nc


def prep_core_inputs(embs_c: np.ndarray, idx6_c: np.ndarray, cap: int):
    """embs_c [n_sh, D] f32; idx6_c [n_sh, S] i64/i32 (col 0 = label -> positive).

    Pairs are bucket-sorted by widx // BK; bucket b's pairs padded to cap with
    idx 0 / e 0. Pair q of bucket b lands at out[p=q%128, slot=q//128]; its
    int16 offset sits at idx_t[b, 16k + q%16, q//16] for all k (8x replicated).
    e is negated for positive slots so every pair contributes softplus(dot)."""
    widx = np.asarray(idx6_c, dtype=np.int64).reshape(-1)          # [NP_]
    samp = np.repeat(np.arange(N_SH, dtype=np.int64), S)
    sign = np.where(np.arange(NP_) % S == 0, -1.0, 1.0).astype(np.float32)
    bucket = widx // BK
    order = np.argsort(bucket, kind="stable")
    widx_s, samp_s, sign_s = widx[order], samp[order], sign[order]
    counts = np.bincount(bucket, minlength=NB)

    idx_pad = np.zeros((NB, cap), np.int16)
    e_pad = np.zeros((NB, cap, D), np.float32)
    pos = 0
    for b in range(NB):
        c = counts[b]
        idx_pad[b, :c] = (widx_s[pos:pos + c] - b * BK).astype(np.int16)
        e_pad[b, :c] = embs_c[samp_s[pos:pos + c]] * sign_s[pos:pos + c, None]
        pos += c

    # idx wrapped in 16 partitions, replicated 8x -> [NB, 128, cap/16]
    iw = idx_pad.reshape(NB, cap // 16, 16).transpose(0, 2, 1)     # [NB,16,ncol]
    idx_t = np.tile(iw, (1, 8, 1))
    # e: pair q=j*128+p -> e_t[b, p, j*128:(j+1)*128]
    e_t = (
        e_pad.reshape(NB, cap // 128, 128, D)
        .transpose(0, 2, 1, 3)
        .reshape(NB, P, cap)
    )
    return np.ascontiguousarray(e_t), np.ascontiguousarray(idx_t)


def kernel(input, embs, label, negs, weights):
    global LAST_EXEC_NS, LAST_NC, LAST_CAP
    embs = np.ascontiguousarray(np.asarray(embs, dtype=np.float32))
    weights = np.ascontiguousarray(np.asarray(weights, dtype=np.float32))
    idx6 = np.empty((N, S), np.int64)
    idx6[:, 0] = np.asarray(label, dtype=np.int64)
    idx6[:, 1:] = np.asarray(negs, dtype=np.int64)

    max_cnt = 0
    for c in range(N_CORES):
        sl = slice(c * N_SH, (c + 1) * N_SH)
        cnt = np.bincount(idx6[sl].reshape(-1) // BK, minlength=NB)
        max_cnt = max(max_cnt, int(cnt.max()))
    cap = -(-max_cnt // 128) * 128

    in_maps = []
    for c in range(N_CORES):
        sl = slice(c * N_SH, (c + 1) * N_SH)
        e_t, idx_t = prep_core_inputs(embs[sl], idx6[sl], cap)
        in_maps.append({"weights": weights, "e_t": e_t, "idx_t": idx_t})

    if LAST_NC is None or LAST_CAP != cap:
        LAST_NC = build_nc(cap)
        LAST_CAP = cap
    nc = LAST_NC
    res = run_bass_kernel_spmd(nc, in_maps, core_ids=list(range(N_CORES)), trace=TRACE)
    if TRACE:
        LAST_EXEC_NS = res.exec_time_ns

    acc_sum = sum(float(np.asarray(r["out"], dtype=np.float64).sum()) for r in res.results)
    n_pads = N_CORES * (NB * cap - NP_)
    total = -acc_sum - n_pads * np.log(2.0)
    return np.float32(total / N)


# revision 17
# speedup vs baseline: 1.4714x; 1.4714x over previous
import sys

sys.path.insert(0, "/opt/trn_rl_repo")

import numpy as np

from concourse import bacc, mybir
from concourse.tile import TileContext
from concourse.bass_utils import run_bass_kernel_spmd

P = 128          # SBUF partitions
D = 128          # embedding dim
S = 6            # 1 positive + 5 negative slots per sample
N = 65536        # total samples
V = 1_000_000    # weights table rows
N_CORES = 8
N_SH = N // N_CORES            # 8192 samples per core
NP_ = N_SH * S                 # 49152 (sample, slot) pairs per core
BK = 32768                     # weight-row bucket size (int16 offset range)
NB = (V + BK - 1) // BK        # 31 buckets

TRACE = False
LAST_EXEC_NS = None
LAST_NC = None
LAST_CAP = None


def build_nc(cap: int):
    """cap: padded pairs per bucket (multiple of 128), same for all buckets."""
    nslot = cap // 128
    ncol = cap // 16
    nc = bacc.Bacc("TRN2", num_swdge_queues=4)
    w_dram = nc.declare_dram_parameter("weights", [V, D], mybir.dt.float32, isOutput=False)
    e_dram = nc.declare_dram_parameter("e_t", [NB, P, cap], mybir.dt.float32, isOutput=False)
    i_dram = nc.declare_dram_parameter("idx_t", [NB, P, ncol], mybir.dt.int16, isOutput=False)
    o_dram = nc.declare_dram_parameter("out", [P, NB], mybir.dt.float32, isOutput=True)

    dma_engines = None  # filled inside context

    with TileContext(nc) as tc:
        with (
            tc.tile_pool(name="ip", bufs=6) as ip,
            tc.tile_pool(name="wp", bufs=6) as wp,
            tc.tile_pool(name="ep", bufs=6) as ep,
            tc.tile_pool(name="sp", bufs=6) as sp,
            tc.tile_pool(name="accp", bufs=1) as accp,
        ):
            dma_engines = [nc.sync, nc.scalar]
            acc = accp.tile([P, NB], mybir.dt.float32)
            for b in range(NB):
                rows = min(BK, V - b * BK)
                it = ip.tile([P, ncol], mybir.dt.int16)
                nc.sync.dma_start(out=it[:], in_=i_dram[b, :, :])
                et = ep.tile([P, cap], mybir.dt.float32)
                dma_engines[b % 2].dma_start(out=et[:], in_=e_dram[b, :, :])
                wt = wp.tile([P, cap], mybir.dt.float32)
                nc.gpsimd.dma_gather(
                    out_ap=wt[:].rearrange("p (j d) -> p j d", d=D),
                    in_ap=w_dram[b * BK: b * BK + rows, :],
                    idxs_ap=it[:],
                    num_idxs=cap, num_idxs_reg=cap, elem_size=D,
                    queue_num=b % 4, single_packet=False,
                )
                prod = sp.tile([P, cap], mybir.dt.float32)
                nc.vector.tensor_tensor(
                    out=prod[:], in0=wt[:], in1=et[:], op=mybir.AluOpType.mult,
                )
                logits = sp.tile([P, nslot], mybir.dt.float32)
                nc.vector.tensor_reduce(
                    out=logits[:],
                    in_=prod[:].rearrange("p (j d) -> p j d", d=D),
                    op=mybir.AluOpType.add, axis=mybir.AxisListType.X,
                )
                # softplus(x) = -ln(sigmoid(-x)); host negates the summed output.
                sg = sp.tile([P, nslot], mybir.dt.float32)
                nc.scalar.activation(
                    out=sg[:], in_=logits[:],
                    func=mybir.ActivationFunctionType.Sigmoid, scale=-1.0,
                )
                ln = sp.tile([P, nslot], mybir.dt.float32)
                nc.scalar.activation(
                    out=ln[:], in_=sg[:],
                    func=mybir.ActivationFunctionType.Ln, scale=1.0,
                    accum_out=acc[:, b:b + 1],
                )
            nc.sync.dma_start(out=o_dram[:, :], in_=acc[:])
    nc.compile()
    return nc


def prep_core_inputs(embs_c: np.ndarray, idx6_c: np.ndarray, cap: int):
    """embs_c [n_sh, D] f32; idx6_c [n_sh, S] i64/i32 (col 0 = label -> positive).

    Pairs are bucket-sorted by widx // BK; bucket b's pairs padded to cap with
    idx 0 / e 0. Pair q of bucket b lands at out[p=q%128, slot=q//128]; its
    int16 offset sits at idx_t[b, 16k + q%16, q//16] for all k (8x replicated).
    e is negated for positive slots so every pair contributes softplus(dot)."""
    widx = np.asarray(idx6_c, dtype=np.int64).reshape(-1)          # [NP_]
    samp = np.repeat(np.arange(N_SH, dtype=np.int64), S)
    sign = np.where(np.arange(NP_) % S == 0, -1.0, 1.0).astype(np.float32)
    bucket = widx // BK
    order = np.argsort(bucket, kind="stable")
    widx_s, samp_s, sign_s = widx[order], samp[order], sign[order]
    counts = np.bincount(bucket, minlength=NB)

    idx_pad = np.zeros((NB, cap), np.int16)
    e_pad = np.zeros((NB, cap, D), np.float32)
    pos = 0
    for b in range(NB):
        c = counts[b]
        idx_pad[b, :c] = (widx_s[pos:pos + c] - b * BK).astype(np.int16)
        e_pad[b, :c] = embs_c[samp_s[pos:pos + c]] * sign_s[pos:pos + c, None]
        pos += c

    # idx wrapped in 16 partitions, replicated 8x -> [NB, 128, cap/16]
    iw = idx_pad.reshape(NB, cap // 16, 16).transpose(0, 2, 1)     # [NB,16,ncol]
    idx_t = np.tile(iw, (1, 8, 1))
    # e: pair q=j*128+p -> e_t[b, p, j*128:(j+1)*128]
    e_t = (
        e_pad.reshape(NB, cap // 128, 128, D)
        .transpose(0, 2, 1, 3)
        .reshape(NB, P, cap)
    )
    return np.ascontiguousarray(e_t), np.ascontiguousarray(idx_t)


def kernel(input, embs, label, negs, weights):
    global LAST_EXEC_NS, LAST_NC, LAST_CAP
    embs = np.ascontiguousarray(np.asarray(embs, dtype=np.float32))
    weights = np.ascontiguousarray(np.asarray(weights, dtype=np.float32))
    idx6 = np.empty((N, S), np.int64)
    idx6[:, 0] = np.asarray(label, dtype=np.int64)
    idx6[:, 1:] = np.asarray(negs, dtype=np.int64)

    max_cnt = 0
    for c in range(N_CORES):
        sl = slice(c * N_SH, (c + 1) * N_SH)
        cnt = np.bincount(idx6[sl].reshape(-1) // BK, minlength=NB)
        max_cnt = max(max_cnt, int(cnt.max()))
    cap = -(-max_cnt // 128) * 128

    in_maps = []
    for c in range(N_CORES):
        sl = slice(c * N_SH, (c + 1) * N_SH)
        e_t, idx_t = prep_core_inputs(embs[sl], idx6[sl], cap)
        in_maps.append({"weights": weights, "e_t": e_t, "idx_t": idx_t})

    if LAST_NC is None or LAST_CAP != cap:
        LAST_NC = build_nc(cap)
        LAST_CAP = cap
    nc = LAST_NC
    res = run_bass_kernel_spmd(nc, in_maps, core_ids=list(range(N_CORES)), trace=TRACE)
    if TRACE:
        LAST_EXEC_NS = res.exec_time_ns

    acc_sum = sum(float(np.asarray(r["out"], dtype=np.float64).sum()) for r in res.results)
    n_pads = N_CORES * (NB * cap - NP_)
    total = -acc_sum - n_pads * np.log(2.0)
    return np.float32(total / N)
